# revision 20
# baseline (speedup 1.0000x reference)
"""Trainium2 Bass kernel for nn_AdaptiveComputationGraph (moe_routing).

Strategy
--------
The router (tiny scalar MLP on normalized per-sample uncertainty) is computed
on host in numpy (sub-millisecond, ~10 MFLOP vs 275 GFLOP of layer compute).
The heavy compute -- the depth-4 chain of [B,D]@[D,D] GEMMs + GELU -- runs
dense on all 8 NeuronCores, data-parallel over the batch (1024 rows/core),
replicated weights, zero cross-core communication.

The device kernel keeps activations FEATURE-MAJOR (transposed, [D, rows]) so
every layer is out^T = gelu(W^T @ in^T + b) with the contraction dim on SBUF
partitions for both operands -- no transposes anywhere in the chain.  Compute
dtype is bf16 (4x TensorE rate vs fp32), fp32 PSUM accumulation, gelu+bias
fused into the PSUM->SBUF eviction on ScalarE.

The kernel outputs the level-1, level-2 and level-4 activations (h1, h2, h4)
for every row; the host selects per row according to the routing level and
re-transposes.  This is correct for any routing distribution and
FLOP-minimal for the actual one (routing sends 8191/8192 samples to the
deepest level, so per-sample early exit would save <0.01% FLOPs).
"""

import sys

sys.path.insert(0, "/opt/trn_rl_repo")

import numpy as np
import ml_dtypes


def _ensure_ntff_hook():
    """concourse's axon trace path reads antenv.axon_hooks, which this image's
    antenv package lacks.  Install an equivalent shim backed by the ctypes
    NTFF driver from trn_agent_boot so trace=True / BASS_TRACE=1 can profile.
    No-op if the real module (or a previous shim) is importable."""
    try:
        import antenv.axon_hooks  # noqa: F401
        return
    except ImportError:
        pass
    try:
        import types
        import antenv
        from trn_agent_boot.trn_boot import _ntff_profile_via_ctypes

        hook = _ntff_profile_via_ctypes("/opt/axon/libaxon_pjrt.so")
        mod = types.ModuleType("antenv.axon_hooks")
        mod._hook = hook
        mod.get_axon_ntff_profile_hook = lambda: mod._hook
        mod.set_axon_ntff_profile_hook = lambda h: setattr(mod, "_hook", h)
        sys.modules["antenv.axon_hooks"] = mod
        antenv.axon_hooks = mod
    except Exception:
        pass


_ensure_ntff_hook()

N_CORES = 8
B, D, L = 8192, 2048, 4
R = B // N_CORES  # rows per core
P = 128  # SBUF partitions
KT = D // P  # contraction tiles per layer
MT = D // P  # output-feature blocks per layer
RC = 512  # rows per matmul (PSUM bank = 512 fp32)
NRC = R // RC

_COMPILED = None
LAST_RESULTS = None  # BassKernelResults of the most recent run (for profiling)


def _build(warmup=True, interleave=True, kmaj0=False):
    from concourse import bacc, mybir, tile

    nc = bacc.Bacc("TRN2", target_bir_lowering=False, debug=False,
                   num_devices=N_CORES)
    bf16 = mybir.dt.bfloat16
    f32 = mybir.dt.float32
    gelu = mybir.ActivationFunctionType.Gelu_apprx_tanh

    xt_ext = nc.declare_dram_parameter("xt", [D, R], bf16, isOutput=False)
    w_ext = nc.declare_dram_parameter("w", [L, D, D], bf16, isOutput=False)
    br_ext = nc.declare_dram_parameter("br", [L, P, MT], f32, isOutput=False)
    h1_ext = nc.declare_dram_parameter("h1", [D, R], bf16, isOutput=True)
    h2_ext = nc.declare_dram_parameter("h2", [D, R], bf16, isOutput=True)
    h4_ext = nc.declare_dram_parameter("h4", [D, R], bf16, isOutput=True)
    outs = {0: h1_ext, 1: h2_ext, 3: h4_ext}

    with tile.TileContext(nc) as tc:
        with (
            tc.tile_pool(name="acts", bufs=48) as act_pool,
            tc.tile_pool(name="wpool", bufs=40) as w_pool,
            tc.tile_pool(name="bias", bufs=4) as b_pool,
            tc.tile_pool(name="psum", bufs=8, space="PSUM") as psum_pool,
        ):
            # Weight DMAs are split into column halves so a single tile's
            # load parallelizes across DMA queues and the first m-blocks
            # only wait on their own half -- cuts the startup stall before
            # the first PSUM group can complete.
            NG = {0: 2, 1: 2, 2: 2, 3: 2}  # column groups per layer

            def load_weights(l):
                ng = NG[l]
                gw = D // ng  # columns per group
                tiles = [[None] * ng for _ in range(KT)]
                for g in range(ng):
                    for k in range(KT):
                        wt = w_pool.tile([P, gw], bf16,
                                         name=f"w{l}_{k}_g{g}", tag="w")
                        nc.sync.dma_start(
                            out=wt[:],
                            in_=w_ext[l, k * P:(k + 1) * P, g * gw:(g + 1) * gw],
                        )
                        tiles[k][g] = wt
                return tiles

            if warmup:
                # PE warmup: the first ~15us are DMA-bound (x + layer-0
                # weights in flight) and the PE_HAM clock gate holds a cold
                # PE at 1.2 GHz until it has been busy ~3.4us.  Run dead
                # matmuls on a zeroed tile into a scratch PSUM bank so the
                # PE enters the real work already at 2.4 GHz.  These execute
                # while the PE would otherwise idle.
                warm_in = act_pool.tile([P, RC], bf16, name="warm_in",
                                        tag="act")
                nc.vector.memset(warm_in[:], 0.0)
                warm_ps = psum_pool.tile([P, RC], f32, name="warm_ps",
                                         tag="ps")
                for i in range(32):
                    nc.tensor.matmul(warm_ps[:], warm_in[:, 0:P], warm_in[:],
                                     start=True, stop=True)

            cur = []
            wts0 = [[None] * NG[0] for _ in range(KT)]
            if interleave:
                # issue (x[k], W0-half0[k]) pairs so the layer-0 critical
                # DMAs lead every queue, then the remaining layer-0 groups.
                gw0 = D // NG[0]
                for k in range(KT):
                    t = act_pool.tile([P, R], bf16, name=f"x{k}", tag="act")
                    nc.sync.dma_start(out=t[:], in_=xt_ext[k * P:(k + 1) * P, :])
                    cur.append(t)
                    wt = w_pool.tile([P, gw0], bf16, name=f"w0_{k}_g0", tag="w")
                    nc.sync.dma_start(out=wt[:],
                                      in_=w_ext[0, k * P:(k + 1) * P, 0:gw0])
                    wts0[k][0] = wt
                for g in range(1, NG[0]):
                    for k in range(KT):
                        wt = w_pool.tile([P, gw0], bf16,
                                         name=f"w0_{k}_g{g}", tag="w")
                        nc.sync.dma_start(
                            out=wt[:],
                            in_=w_ext[0, k * P:(k + 1) * P, g * gw0:(g + 1) * gw0],
                        )
                        wts0[k][g] = wt
            else:
                for k in range(KT):
                    t = act_pool.tile([P, R], bf16, name=f"x{k}", tag="act")
                    nc.sync.dma_start(out=t[:], in_=xt_ext[k * P:(k + 1) * P, :])
                    cur.append(t)
                wts0 = None

            bias_tiles = []
            for l in range(L):
                bt = b_pool.tile([P, MT], f32, name=f"bias{l}", tag="bias")
                nc.sync.dma_start(out=bt[:], in_=br_ext[l])
                bias_tiles.append(bt)

            def evict_block(l, m, psums, nxt):
                ot = act_pool.tile([P, R], bf16, name=f"h{l}_{m}", tag="act")
                for r in range(NRC):
                    nc.scalar.activation(
                        ot[:, r * RC:(r + 1) * RC], psums[r][:], gelu,
                        bias=bias_tiles[l][:, m:m + 1],
                    )
                if l in outs:
                    nc.sync.dma_start(
                        out=outs[l][m * P:(m + 1) * P, :], in_=ot[:]
                    )
                nxt.append(ot)

            for l in range(L):
                wts = wts0 if (l == 0 and wts0 is not None) else load_weights(l)
                ng = NG[l]
                mg = MT // ng  # m-blocks per column group
                nxt = []
                m_start = 0
                if l == 0 and kmaj0:
                    # Startup is DMA-bound: x and layer-0 weight tiles land
                    # one (x[k], w[k]) pair at a time (~1.4us apart).  In
                    # m-major order each arriving pair enables only ~0.4us
                    # of matmuls, so the PE idles between arrivals.  Run the
                    # first 4 m-blocks K-MAJOR with all 8 PSUM banks open:
                    # each arriving pair then feeds 4 blocks x 2 row-chunks
                    # = ~1.7us of work and the PE stays dense from the first
                    # arrival.  k still ascends within each PSUM group, so
                    # results are bit-identical to the m-major order.
                    M1 = 4
                    ps1 = [
                        [psum_pool.tile([P, RC], f32, name=f"ps0_{m}_{r}",
                                        tag="ps") for r in range(NRC)]
                        for m in range(M1)
                    ]
                    for k in range(KT):
                        for m in range(M1):
                            wap = wts[k][0][:, m * P:(m + 1) * P]
                            for r in range(NRC):
                                nc.tensor.matmul(
                                    ps1[m][r][:], wap,
                                    cur[k][:, r * RC:(r + 1) * RC],
                                    start=(k == 0), stop=(k == KT - 1),
                                )
                    for m in range(M1):
                        evict_block(l, m, ps1[m], nxt)
                    m_start = M1
                for m in range(m_start, MT):
                    psums = [
                        psum_pool.tile([P, RC], f32, name=f"ps{l}_{m}_{r}", tag="ps")
                        for r in range(NRC)
                    ]
                    g, ml = divmod(m, mg)
                    for k in range(KT):
                        wap = wts[k][g][:, ml * P:(ml + 1) * P]
                        for r in range(NRC):
                            nc.tensor.matmul(
                                psums[r][:], wap, cur[k][:, r * RC:(r + 1) * RC],
                                start=(k == 0), stop=(k == KT - 1),
                            )
                    evict_block(l, m, psums, nxt)
                cur = nxt

    nc.compile()
    return nc


def _get_compiled():
    global _COMPILED
    if _COMPILED is None:
        _COMPILED = _build()
    return _COMPILED


def _route_np(unc, rw1, rb1, rw2, rb2, rw3, rb3):
    """Replicates reference._route in float32 numpy (argmax of softmax ==
    argmax of logits)."""
    unc = unc.astype(np.float32)
    u = (unc - unc.min()) / (unc.max() - unc.min() + np.float32(1e-8))
    h = np.maximum(u[:, None] * rw1[0][None, :] + rb1, np.float32(0))
    h = np.maximum(h @ rw2 + rb2, np.float32(0))
    logits = h @ rw3 + rb3
    return np.argmax(logits, axis=-1)


def kernel(x, current_uncertainty, Ws, bs, rw1, rb1, rw2, rb2, rw3, rb3):
    global LAST_RESULTS
    from concourse.bass_utils import run_bass_kernel_spmd

    x = np.asarray(x, dtype=np.float32)
    Ws = np.asarray(Ws, dtype=np.float32)
    bs = np.asarray(bs, dtype=np.float32)

    routing = _route_np(
        np.asarray(current_uncertainty, dtype=np.float32),
        np.asarray(rw1, dtype=np.float32), np.asarray(rb1, dtype=np.float32),
        np.asarray(rw2, dtype=np.float32), np.asarray(rb2, dtype=np.float32),
        np.asarray(rw3, dtype=np.float32), np.asarray(rb3, dtype=np.float32),
    )

    bf = ml_dtypes.bfloat16
    xt = np.ascontiguousarray(x.T).astype(bf)  # [D, B]
    w_bf = np.ascontiguousarray(Ws).astype(bf)  # [L, D, D]
    # bias rearranged so partition p of feature-block m holds bs[l, m*128+p]
    br = np.ascontiguousarray(
        bs.reshape(L, MT, P).transpose(0, 2, 1)
    ).astype(np.float32)  # [L, P, MT]

    in_maps = [
        {
            "xt": np.ascontiguousarray(xt[:, c * R:(c + 1) * R]),
            "w": w_bf,
            "br": br,
        }
        for c in range(N_CORES)
    ]

    nc = _get_compiled()
    res = run_bass_kernel_spmd(nc, in_maps, list(range(N_CORES)))
    LAST_RESULTS = res

    def gather(name):
        full = np.concatenate(
            [np.asarray(res.results[c][name]) for c in range(N_CORES)], axis=1
        )  # [D, B]
        return np.ascontiguousarray(full.T).astype(np.float32)  # [B, D]

    h1 = gather("h1")
    h2 = gather("h2")
    h4 = gather("h4")

    final = h4
    lvl0 = routing == 0
    lvl1 = routing == 1
    if lvl0.any():
        final[lvl0] = h1[lvl0]
    if lvl1.any():
        final[lvl1] = h2[lvl1]

    mask = routing.astype(np.float32)
    return final, mask


# revision 21
# speedup vs baseline: 1.0054x; 1.0054x over previous
"""Trainium2 Bass kernel for nn_AdaptiveComputationGraph (moe_routing).

Strategy
--------
The router (tiny scalar MLP on normalized per-sample uncertainty) is computed
on host in numpy (sub-millisecond, ~10 MFLOP vs 275 GFLOP of layer compute).
The heavy compute -- the depth-4 chain of [B,D]@[D,D] GEMMs + GELU -- runs
dense on all 8 NeuronCores, data-parallel over the batch (1024 rows/core),
replicated weights, zero cross-core communication.

The device kernel keeps activations FEATURE-MAJOR (transposed, [D, rows]) so
every layer is out^T = gelu(W^T @ in^T + b) with the contraction dim on SBUF
partitions for both operands -- no transposes anywhere in the chain.  Compute
dtype is bf16 (4x TensorE rate vs fp32), fp32 PSUM accumulation, gelu+bias
fused into the PSUM->SBUF eviction on ScalarE.

The kernel outputs the level-1, level-2 and level-4 activations (h1, h2, h4)
for every row; the host selects per row according to the routing level and
re-transposes.  This is correct for any routing distribution and
FLOP-minimal for the actual one (routing sends 8191/8192 samples to the
deepest level, so per-sample early exit would save <0.01% FLOPs).
"""

import sys

sys.path.insert(0, "/opt/trn_rl_repo")

import numpy as np
import ml_dtypes


def _ensure_ntff_hook():
    """concourse's axon trace path reads antenv.axon_hooks, which this image's
    antenv package lacks.  Install an equivalent shim backed by the ctypes
    NTFF driver from trn_agent_boot so trace=True / BASS_TRACE=1 can profile.
    No-op if the real module (or a previous shim) is importable."""
    try:
        import antenv.axon_hooks  # noqa: F401
        return
    except ImportError:
        pass
    try:
        import types
        import antenv
        from trn_agent_boot.trn_boot import _ntff_profile_via_ctypes

        hook = _ntff_profile_via_ctypes("/opt/axon/libaxon_pjrt.so")
        mod = types.ModuleType("antenv.axon_hooks")
        mod._hook = hook
        mod.get_axon_ntff_profile_hook = lambda: mod._hook
        mod.set_axon_ntff_profile_hook = lambda h: setattr(mod, "_hook", h)
        sys.modules["antenv.axon_hooks"] = mod
        antenv.axon_hooks = mod
    except Exception:
        pass


_ensure_ntff_hook()

N_CORES = 8
B, D, L = 8192, 2048, 4
R = B // N_CORES  # rows per core
P = 128  # SBUF partitions
KT = D // P  # contraction tiles per layer
MT = D // P  # output-feature blocks per layer
RC = 512  # rows per matmul (PSUM bank = 512 fp32)
NRC = R // RC

_COMPILED = None
LAST_RESULTS = None  # BassKernelResults of the most recent run (for profiling)


def _build(warmup=True, interleave=True, kmaj0=False):
    from concourse import bacc, mybir, tile

    nc = bacc.Bacc("TRN2", target_bir_lowering=False, debug=False,
                   num_devices=N_CORES)
    bf16 = mybir.dt.bfloat16
    f32 = mybir.dt.float32
    gelu = mybir.ActivationFunctionType.Gelu_apprx_tanh

    xt_ext = nc.declare_dram_parameter("xt", [D, R], bf16, isOutput=False)
    w_ext = nc.declare_dram_parameter("w", [L, D, D], bf16, isOutput=False)
    br_ext = nc.declare_dram_parameter("br", [L, P, MT], f32, isOutput=False)
    h1_ext = nc.declare_dram_parameter("h1", [D, R], bf16, isOutput=True)
    h2_ext = nc.declare_dram_parameter("h2", [D, R], bf16, isOutput=True)
    h4_ext = nc.declare_dram_parameter("h4", [D, R], bf16, isOutput=True)
    outs = {0: h1_ext, 1: h2_ext, 3: h4_ext}

    with tile.TileContext(nc) as tc:
        with (
            tc.tile_pool(name="acts", bufs=48) as act_pool,
            tc.tile_pool(name="wpool", bufs=40) as w_pool,
            tc.tile_pool(name="bias", bufs=4) as b_pool,
            tc.tile_pool(name="psum", bufs=8, space="PSUM") as psum_pool,
        ):
            # Weight DMAs are split into column halves so a single tile's
            # load parallelizes across DMA queues and the first m-blocks
            # only wait on their own half -- cuts the startup stall before
            # the first PSUM group can complete.
            NG = {0: 2, 1: 2, 2: 2, 3: 2}  # column groups per layer

            def load_weights(l):
                ng = NG[l]
                gw = D // ng  # columns per group
                tiles = [[None] * ng for _ in range(KT)]
                for g in range(ng):
                    for k in range(KT):
                        wt = w_pool.tile([P, gw], bf16,
                                         name=f"w{l}_{k}_g{g}", tag="w")
                        nc.sync.dma_start(
                            out=wt[:],
                            in_=w_ext[l, k * P:(k + 1) * P, g * gw:(g + 1) * gw],
                        )
                        tiles[k][g] = wt
                return tiles

            if warmup:
                # PE warmup: the first ~15us are DMA-bound (x + layer-0
                # weights in flight) and the PE_HAM clock gate holds a cold
                # PE at 1.2 GHz until it has been busy ~3.4us.  Run dead
                # matmuls on a zeroed tile into a scratch PSUM bank so the
                # PE enters the real work already at 2.4 GHz.  These execute
                # while the PE would otherwise idle.
                warm_in = act_pool.tile([P, RC], bf16, name="warm_in",
                                        tag="act")
                nc.vector.memset(warm_in[:], 0.0)
                warm_ps = psum_pool.tile([P, RC], f32, name="warm_ps",
                                         tag="ps")
                for i in range(32):
                    nc.tensor.matmul(warm_ps[:], warm_in[:, 0:P], warm_in[:],
                                     start=True, stop=True)

            bias_tiles = []
            for l in range(L):
                bt = b_pool.tile([P, MT], f32, name=f"bias{l}", tag="bias")
                nc.sync.dma_start(out=bt[:], in_=br_ext[l])
                bias_tiles.append(bt)

            cur = []
            wts0 = [[None] * NG[0] for _ in range(KT)]
            if interleave:
                # issue (x[k], W0-half0[k]) pairs so the layer-0 critical
                # DMAs lead every queue, then the remaining layer-0 groups.
                gw0 = D // NG[0]
                for k in range(KT):
                    t = act_pool.tile([P, R], bf16, name=f"x{k}", tag="act")
                    nc.sync.dma_start(out=t[:], in_=xt_ext[k * P:(k + 1) * P, :])
                    cur.append(t)
                    wt = w_pool.tile([P, gw0], bf16, name=f"w0_{k}_g0", tag="w")
                    nc.sync.dma_start(out=wt[:],
                                      in_=w_ext[0, k * P:(k + 1) * P, 0:gw0])
                    wts0[k][0] = wt
                for g in range(1, NG[0]):
                    for k in range(KT):
                        wt = w_pool.tile([P, gw0], bf16,
                                         name=f"w0_{k}_g{g}", tag="w")
                        nc.sync.dma_start(
                            out=wt[:],
                            in_=w_ext[0, k * P:(k + 1) * P, g * gw0:(g + 1) * gw0],
                        )
                        wts0[k][g] = wt
            else:
                for k in range(KT):
                    t = act_pool.tile([P, R], bf16, name=f"x{k}", tag="act")
                    nc.sync.dma_start(out=t[:], in_=xt_ext[k * P:(k + 1) * P, :])
                    cur.append(t)
                wts0 = None

            def evict_block(l, m, psums, nxt):
                ot = act_pool.tile([P, R], bf16, name=f"h{l}_{m}", tag="act")
                for r in range(NRC):
                    nc.scalar.activation(
                        ot[:, r * RC:(r + 1) * RC], psums[r][:], gelu,
                        bias=bias_tiles[l][:, m:m + 1],
                    )
                if l in outs:
                    nc.sync.dma_start(
                        out=outs[l][m * P:(m + 1) * P, :], in_=ot[:]
                    )
                nxt.append(ot)

            for l in range(L):
                wts = wts0 if (l == 0 and wts0 is not None) else load_weights(l)
                ng = NG[l]
                mg = MT // ng  # m-blocks per column group
                nxt = []
                m_start = 0
                if l == 0 and kmaj0:
                    # Startup is DMA-bound: x and layer-0 weight tiles land
                    # one (x[k], w[k]) pair at a time (~1.4us apart).  In
                    # m-major order each arriving pair enables only ~0.4us
                    # of matmuls, so the PE idles between arrivals.  Run the
                    # first 4 m-blocks K-MAJOR with all 8 PSUM banks open:
                    # each arriving pair then feeds 4 blocks x 2 row-chunks
                    # = ~1.7us of work and the PE stays dense from the first
                    # arrival.  k still ascends within each PSUM group, so
                    # results are bit-identical to the m-major order.
                    M1 = 4
                    ps1 = [
                        [psum_pool.tile([P, RC], f32, name=f"ps0_{m}_{r}",
                                        tag="ps") for r in range(NRC)]
                        for m in range(M1)
                    ]
                    for k in range(KT):
                        for m in range(M1):
                            wap = wts[k][0][:, m * P:(m + 1) * P]
                            for r in range(NRC):
                                nc.tensor.matmul(
                                    ps1[m][r][:], wap,
                                    cur[k][:, r * RC:(r + 1) * RC],
                                    start=(k == 0), stop=(k == KT - 1),
                                )
                    for m in range(M1):
                        evict_block(l, m, ps1[m], nxt)
                    m_start = M1
                for m in range(m_start, MT):
                    psums = [
                        psum_pool.tile([P, RC], f32, name=f"ps{l}_{m}_{r}", tag="ps")
                        for r in range(NRC)
                    ]
                    g, ml = divmod(m, mg)
                    for k in range(KT):
                        wap = wts[k][g][:, ml * P:(ml + 1) * P]
                        for r in range(NRC):
                            nc.tensor.matmul(
                                psums[r][:], wap, cur[k][:, r * RC:(r + 1) * RC],
                                start=(k == 0), stop=(k == KT - 1),
                            )
                    evict_block(l, m, psums, nxt)
                cur = nxt

    nc.compile()
    return nc


def _get_compiled():
    global _COMPILED
    if _COMPILED is None:
        _COMPILED = _build()
    return _COMPILED


def _route_np(unc, rw1, rb1, rw2, rb2, rw3, rb3):
    """Replicates reference._route in float32 numpy (argmax of softmax ==
    argmax of logits)."""
    unc = unc.astype(np.float32)
    u = (unc - unc.min()) / (unc.max() - unc.min() + np.float32(1e-8))
    h = np.maximum(u[:, None] * rw1[0][None, :] + rb1, np.float32(0))
    h = np.maximum(h @ rw2 + rb2, np.float32(0))
    logits = h @ rw3 + rb3
    return np.argmax(logits, axis=-1)


def kernel(x, current_uncertainty, Ws, bs, rw1, rb1, rw2, rb2, rw3, rb3):
    global LAST_RESULTS
    from concourse.bass_utils import run_bass_kernel_spmd

    x = np.asarray(x, dtype=np.float32)
    Ws = np.asarray(Ws, dtype=np.float32)
    bs = np.asarray(bs, dtype=np.float32)

    routing = _route_np(
        np.asarray(current_uncertainty, dtype=np.float32),
        np.asarray(rw1, dtype=np.float32), np.asarray(rb1, dtype=np.float32),
        np.asarray(rw2, dtype=np.float32), np.asarray(rb2, dtype=np.float32),
        np.asarray(rw3, dtype=np.float32), np.asarray(rb3, dtype=np.float32),
    )

    bf = ml_dtypes.bfloat16
    xt = np.ascontiguousarray(x.T).astype(bf)  # [D, B]
    w_bf = np.ascontiguousarray(Ws).astype(bf)  # [L, D, D]
    # bias rearranged so partition p of feature-block m holds bs[l, m*128+p]
    br = np.ascontiguousarray(
        bs.reshape(L, MT, P).transpose(0, 2, 1)
    ).astype(np.float32)  # [L, P, MT]

    in_maps = [
        {
            "xt": np.ascontiguousarray(xt[:, c * R:(c + 1) * R]),
            "w": w_bf,
            "br": br,
        }
        for c in range(N_CORES)
    ]

    nc = _get_compiled()
    res = run_bass_kernel_spmd(nc, in_maps, list(range(N_CORES)))
    LAST_RESULTS = res

    def gather(name):
        full = np.concatenate(
            [np.asarray(res.results[c][name]) for c in range(N_CORES)], axis=1
        )  # [D, B]
        return np.ascontiguousarray(full.T).astype(np.float32)  # [B, D]

    h1 = gather("h1")
    h2 = gather("h2")
    h4 = gather("h4")

    final = h4
    lvl0 = routing == 0
    lvl1 = routing == 1
    if lvl0.any():
        final[lvl0] = h1[lvl0]
    if lvl1.any():
        final[lvl1] = h2[lvl1]

    mask = routing.astype(np.float32)
    return final, mask


# revision 22
# speedup vs baseline: 1.0099x; 1.0045x over previous
"""Trainium2 Bass kernel for nn_AdaptiveComputationGraph (moe_routing).

Strategy
--------
The router (tiny scalar MLP on normalized per-sample uncertainty) is computed
on host in numpy (sub-millisecond, ~10 MFLOP vs 275 GFLOP of layer compute).
The heavy compute -- the depth-4 chain of [B,D]@[D,D] GEMMs + GELU -- runs
dense on all 8 NeuronCores, data-parallel over the batch (1024 rows/core),
replicated weights, zero cross-core communication.

The device kernel keeps activations FEATURE-MAJOR (transposed, [D, rows]) so
every layer is out^T = gelu(W^T @ in^T + b) with the contraction dim on SBUF
partitions for both operands -- no transposes anywhere in the chain.  Compute
dtype is bf16 (4x TensorE rate vs fp32), fp32 PSUM accumulation, gelu+bias
fused into the PSUM->SBUF eviction on ScalarE.

The kernel outputs the level-1, level-2 and level-4 activations (h1, h2, h4)
for every row; the host selects per row according to the routing level and
re-transposes.  This is correct for any routing distribution and
FLOP-minimal for the actual one (routing sends 8191/8192 samples to the
deepest level, so per-sample early exit would save <0.01% FLOPs).
"""

import sys

sys.path.insert(0, "/opt/trn_rl_repo")

import numpy as np
import ml_dtypes


def _ensure_ntff_hook():
    """concourse's axon trace path reads antenv.axon_hooks, which this image's
    antenv package lacks.  Install an equivalent shim backed by the ctypes
    NTFF driver from trn_agent_boot so trace=True / BASS_TRACE=1 can profile.
    No-op if the real module (or a previous shim) is importable."""
    try:
        import antenv.axon_hooks  # noqa: F401
        return
    except ImportError:
        pass
    try:
        import types
        import antenv
        from trn_agent_boot.trn_boot import _ntff_profile_via_ctypes

        hook = _ntff_profile_via_ctypes("/opt/axon/libaxon_pjrt.so")
        mod = types.ModuleType("antenv.axon_hooks")
        mod._hook = hook
        mod.get_axon_ntff_profile_hook = lambda: mod._hook
        mod.set_axon_ntff_profile_hook = lambda h: setattr(mod, "_hook", h)
        sys.modules["antenv.axon_hooks"] = mod
        antenv.axon_hooks = mod
    except Exception:
        pass


_ensure_ntff_hook()

N_CORES = 8
B, D, L = 8192, 2048, 4
R = B // N_CORES  # rows per core
P = 128  # SBUF partitions
KT = D // P  # contraction tiles per layer
MT = D // P  # output-feature blocks per layer
RC = 512  # rows per matmul (PSUM bank = 512 fp32)
NRC = R // RC

_COMPILED = None
LAST_RESULTS = None  # BassKernelResults of the most recent run (for profiling)


def _build(warmup=True, interleave=True, kmaj0=False):
    from concourse import bacc, mybir, tile

    nc = bacc.Bacc("TRN2", target_bir_lowering=False, debug=False,
                   num_devices=N_CORES)
    bf16 = mybir.dt.bfloat16
    f32 = mybir.dt.float32
    gelu = mybir.ActivationFunctionType.Gelu_apprx_tanh

    xt_ext = nc.declare_dram_parameter("xt", [D, R], bf16, isOutput=False)
    w_ext = nc.declare_dram_parameter("w", [L, D, D], bf16, isOutput=False)
    br_ext = nc.declare_dram_parameter("br", [L, P, MT], f32, isOutput=False)
    h1_ext = nc.declare_dram_parameter("h1", [D, R], bf16, isOutput=True)
    h2_ext = nc.declare_dram_parameter("h2", [D, R], bf16, isOutput=True)
    h4_ext = nc.declare_dram_parameter("h4", [D, R], bf16, isOutput=True)
    outs = {0: h1_ext, 1: h2_ext, 3: h4_ext}

    with tile.TileContext(nc) as tc:
        with (
            tc.tile_pool(name="acts", bufs=48) as act_pool,
            tc.tile_pool(name="wpool", bufs=40) as w_pool,
            tc.tile_pool(name="bias", bufs=4) as b_pool,
            tc.tile_pool(name="psum", bufs=8, space="PSUM") as psum_pool,
        ):
            # Weight DMAs are split into column halves so a single tile's
            # load parallelizes across DMA queues and the first m-blocks
            # only wait on their own half -- cuts the startup stall before
            # the first PSUM group can complete.
            NG = {0: 2, 1: 2, 2: 2, 3: 2}  # column groups per layer

            def load_weights(l):
                ng = NG[l]
                gw = D // ng  # columns per group
                tiles = [[None] * ng for _ in range(KT)]
                for g in range(ng):
                    for k in range(KT):
                        wt = w_pool.tile([P, gw], bf16,
                                         name=f"w{l}_{k}_g{g}", tag="w")
                        nc.sync.dma_start(
                            out=wt[:],
                            in_=w_ext[l, k * P:(k + 1) * P, g * gw:(g + 1) * gw],
                        )
                        tiles[k][g] = wt
                return tiles

            if warmup:
                # PE warmup: the first ~15us are DMA-bound (x + layer-0
                # weights in flight) and the PE_HAM clock gate holds a cold
                # PE at 1.2 GHz until it has been busy ~3.4us.  Run dead
                # matmuls on a zeroed tile into a scratch PSUM bank so the
                # PE enters the real work already at 2.4 GHz.  These execute
                # while the PE would otherwise idle.
                warm_in = act_pool.tile([P, RC], bf16, name="warm_in",
                                        tag="act")
                nc.vector.memset(warm_in[:], 0.0)
                warm_ps = psum_pool.tile([P, RC], f32, name="warm_ps",
                                         tag="ps")
                for i in range(32):
                    nc.tensor.matmul(warm_ps[:], warm_in[:, 0:P], warm_in[:],
                                     start=True, stop=True)

            bias_tiles = []
            for l in range(L):
                bt = b_pool.tile([P, MT], f32, name=f"bias{l}", tag="bias")
                nc.sync.dma_start(out=bt[:], in_=br_ext[l])
                bias_tiles.append(bt)

            cur = []
            wts0 = [[None] * NG[0] for _ in range(KT)]
            if interleave:
                # issue (x[k], W0-half0[k]) pairs so the layer-0 critical
                # DMAs lead every queue, then the remaining layer-0 groups.
                gw0 = D // NG[0]
                H = P // 2
                for k in range(KT):
                    t = act_pool.tile([P, R], bf16, name=f"x{k}", tag="act")
                    wt = w_pool.tile([P, gw0], bf16, name=f"w0_{k}_g0", tag="w")
                    if k < 4:
                        # the first k-tiles gate the PE's first real matmuls
                        # (measured stalls at ts~18-21us): partition-split
                        # them across two queues (contiguous on both sides)
                        # so they land in half the time
                        nc.sync.dma_start(out=t[0:H, :],
                                          in_=xt_ext[k * P:k * P + H, :])
                        nc.sync.dma_start(out=t[H:P, :],
                                          in_=xt_ext[k * P + H:(k + 1) * P, :])
                        nc.sync.dma_start(out=wt[0:H, :],
                                          in_=w_ext[0, k * P:k * P + H, 0:gw0])
                        nc.sync.dma_start(out=wt[H:P, :],
                                          in_=w_ext[0, k * P + H:(k + 1) * P, 0:gw0])
                    else:
                        nc.sync.dma_start(out=t[:],
                                          in_=xt_ext[k * P:(k + 1) * P, :])
                        nc.sync.dma_start(out=wt[:],
                                          in_=w_ext[0, k * P:(k + 1) * P, 0:gw0])
                    cur.append(t)
                    wts0[k][0] = wt
                for g in range(1, NG[0]):
                    for k in range(KT):
                        wt = w_pool.tile([P, gw0], bf16,
                                         name=f"w0_{k}_g{g}", tag="w")
                        nc.sync.dma_start(
                            out=wt[:],
                            in_=w_ext[0, k * P:(k + 1) * P, g * gw0:(g + 1) * gw0],
                        )
                        wts0[k][g] = wt
            else:
                for k in range(KT):
                    t = act_pool.tile([P, R], bf16, name=f"x{k}", tag="act")
                    nc.sync.dma_start(out=t[:], in_=xt_ext[k * P:(k + 1) * P, :])
                    cur.append(t)
                wts0 = None

            def evict_block(l, m, psums, nxt):
                ot = act_pool.tile([P, R], bf16, name=f"h{l}_{m}", tag="act")
                for r in range(NRC):
                    nc.scalar.activation(
                        ot[:, r * RC:(r + 1) * RC], psums[r][:], gelu,
                        bias=bias_tiles[l][:, m:m + 1],
                    )
                if l in outs:
                    nc.sync.dma_start(
                        out=outs[l][m * P:(m + 1) * P, :], in_=ot[:]
                    )
                nxt.append(ot)

            for l in range(L):
                wts = wts0 if (l == 0 and wts0 is not None) else load_weights(l)
                ng = NG[l]
                mg = MT // ng  # m-blocks per column group
                nxt = []
                m_start = 0
                if l == 0 and kmaj0:
                    # Startup is DMA-bound: x and layer-0 weight tiles land
                    # one (x[k], w[k]) pair at a time (~1.4us apart).  In
                    # m-major order each arriving pair enables only ~0.4us
                    # of matmuls, so the PE idles between arrivals.  Run the
                    # first 4 m-blocks K-MAJOR with all 8 PSUM banks open:
                    # each arriving pair then feeds 4 blocks x 2 row-chunks
                    # = ~1.7us of work and the PE stays dense from the first
                    # arrival.  k still ascends within each PSUM group, so
                    # results are bit-identical to the m-major order.
                    M1 = 4
                    ps1 = [
                        [psum_pool.tile([P, RC], f32, name=f"ps0_{m}_{r}",
                                        tag="ps") for r in range(NRC)]
                        for m in range(M1)
                    ]
                    for k in range(KT):
                        for m in range(M1):
                            wap = wts[k][0][:, m * P:(m + 1) * P]
                            for r in range(NRC):
                                nc.tensor.matmul(
                                    ps1[m][r][:], wap,
                                    cur[k][:, r * RC:(r + 1) * RC],
                                    start=(k == 0), stop=(k == KT - 1),
                                )
                    for m in range(M1):
                        evict_block(l, m, ps1[m], nxt)
                    m_start = M1
                for m in range(m_start, MT):
                    psums = [
                        psum_pool.tile([P, RC], f32, name=f"ps{l}_{m}_{r}", tag="ps")
                        for r in range(NRC)
                    ]
                    g, ml = divmod(m, mg)
                    for k in range(KT):
                        wap = wts[k][g][:, ml * P:(ml + 1) * P]
                        for r in range(NRC):
                            nc.tensor.matmul(
                                psums[r][:], wap, cur[k][:, r * RC:(r + 1) * RC],
                                start=(k == 0), stop=(k == KT - 1),
                            )
                    evict_block(l, m, psums, nxt)
                cur = nxt

    nc.compile()
    return nc


def _get_compiled():
    global _COMPILED
    if _COMPILED is None:
        _COMPILED = _build()
    return _COMPILED


def _route_np(unc, rw1, rb1, rw2, rb2, rw3, rb3):
    """Replicates reference._route in float32 numpy (argmax of softmax ==
    argmax of logits)."""
    unc = unc.astype(np.float32)
    u = (unc - unc.min()) / (unc.max() - unc.min() + np.float32(1e-8))
    h = np.maximum(u[:, None] * rw1[0][None, :] + rb1, np.float32(0))
    h = np.maximum(h @ rw2 + rb2, np.float32(0))
    logits = h @ rw3 + rb3
    return np.argmax(logits, axis=-1)


def kernel(x, current_uncertainty, Ws, bs, rw1, rb1, rw2, rb2, rw3, rb3):
    global LAST_RESULTS
    from concourse.bass_utils import run_bass_kernel_spmd

    x = np.asarray(x, dtype=np.float32)
    Ws = np.asarray(Ws, dtype=np.float32)
    bs = np.asarray(bs, dtype=np.float32)

    routing = _route_np(
        np.asarray(current_uncertainty, dtype=np.float32),
        np.asarray(rw1, dtype=np.float32), np.asarray(rb1, dtype=np.float32),
        np.asarray(rw2, dtype=np.float32), np.asarray(rb2, dtype=np.float32),
        np.asarray(rw3, dtype=np.float32), np.asarray(rb3, dtype=np.float32),
    )

    bf = ml_dtypes.bfloat16
    xt = np.ascontiguousarray(x.T).astype(bf)  # [D, B]
    w_bf = np.ascontiguousarray(Ws).astype(bf)  # [L, D, D]
    # bias rearranged so partition p of feature-block m holds bs[l, m*128+p]
    br = np.ascontiguousarray(
        bs.reshape(L, MT, P).transpose(0, 2, 1)
    ).astype(np.float32)  # [L, P, MT]

    in_maps = [
        {
            "xt": np.ascontiguousarray(xt[:, c * R:(c + 1) * R]),
            "w": w_bf,
            "br": br,
        }
        for c in range(N_CORES)
    ]

    nc = _get_compiled()
    res = run_bass_kernel_spmd(nc, in_maps, list(range(N_CORES)))
    LAST_RESULTS = res

    def gather(name):
        full = np.concatenate(
            [np.asarray(res.results[c][name]) for c in range(N_CORES)], axis=1
        )  # [D, B]
        return np.ascontiguousarray(full.T).astype(np.float32)  # [B, D]

    h1 = gather("h1")
    h2 = gather("h2")
    h4 = gather("h4")

    final = h4
    lvl0 = routing == 0
    lvl1 = routing == 1
    if lvl0.any():
        final[lvl0] = h1[lvl0]
    if lvl1.any():
        final[lvl1] = h2[lvl1]

    mask = routing.astype(np.float32)
    return final, mask
